# revision 1
# baseline (speedup 1.0000x reference)
"""Trainium2 Bass kernel for nn_Attention_layer (dense_transformer).

Per batch element b (one NeuronCore each, 8 cores):
  k = wk @ x + bk;  q = wq @ x + bq;  v = wv @ x + bv      (1x1x1 conv)
  per (h,w):  scores^T[j,i] = (q_hw^T k_hw) * 1/sqrt(S)    (contract S=128)
              a^T = softmax over i (free axis, skip-max)    via ACT Exp+accum_out
              att[s,j] = v_hw @ a_hw                        (PE transposes for v^T, a)
  out = wo @ att + bo + x                                   (residual via DMA accum)

pos = d*256 + hw (d-major).  SBUF big tiles are tag-chained in one pool so
attention-phase tensors reuse the x-tile slots after the projections finish.
"""

import numpy as np
import ml_dtypes

import concourse.bacc as bacc
import concourse.tile as tile
from concourse import mybir
from concourse.bass_utils import run_bass_kernel_spmd

F32 = mybir.dt.float32
BF16 = mybir.dt.bfloat16
AF = mybir.ActivationFunctionType

B, C, S, D, H, W = 8, 256, 128, 64, 16, 16
HW = H * W            # 256
NPOS = D * HW         # 16384
NCHUNK = NPOS // 512  # 32
SCALE = float(1.0 / np.sqrt(np.float32(S)))

CFG = {
    "resid_dma_accum": False,  # residual add via DMA accumulate (else gpsimd)
    "loop_n": 1,               # on-device repeats of the whole body (timing)
    "trace": False,
}

_CACHE = {}


def _emit(nc, tc, io, ctx):
    xb, xf, wkT, wqT, wvT, woT, bk, bq, bv, bo, ident, boT, ones, out_d = io

    big = ctx.enter_context(tc.tile_pool(name="big", bufs=5))
    med = ctx.enter_context(tc.tile_pool(name="med", bufs=1))
    ring = ctx.enter_context(tc.tile_pool(name="ring", bufs=4))
    oring = ctx.enter_context(tc.tile_pool(name="oring", bufs=3))
    xring = ctx.enter_context(tc.tile_pool(name="xring", bufs=6))
    pool = ctx.enter_context(tc.tile_pool(name="const", bufs=1))
    pp_mm = ctx.enter_context(tc.tile_pool(name="pp_mm", bufs=4, space="PSUM"))
    pp_et = ctx.enter_context(tc.tile_pool(name="pp_et", bufs=2, space="PSUM"))
    pp_tr = ctx.enter_context(tc.tile_pool(name="pp_tr", bufs=2, space="PSUM"))
    pp_at = pp_mm

    # ---- constants ------------------------------------------------------
    id_sb = pool.tile([128, 128], BF16, tag="ident")
    nc.sync.dma_start(id_sb[:], ident[:])
    w_sb = {}
    for nm, t in (("wk", wkT), ("wq", wqT), ("wv", wvT)):
        for h in range(2):
            w_sb[nm, h] = pool.tile([128, 128], BF16, tag=f"w_{nm}{h}", name=f"w_{nm}{h}")
            nc.sync.dma_start(w_sb[nm, h][:], t[h * 128:(h + 1) * 128, :])
    woT_sb = pool.tile([128, 256], BF16, tag="woT")
    nc.sync.dma_start(woT_sb[:], woT[:])
    b_sb = {}
    for nm, t in (("bk", bk), ("bq", bq), ("bv", bv)):
        b_sb[nm] = pool.tile([128, 1], F32, tag=f"b_{nm}", name=f"b_{nm}")
        nc.sync.dma_start(b_sb[nm][:], t[:])
    boT_sb = pool.tile([1, C], BF16, tag="boT")
    nc.sync.dma_start(boT_sb[:], boT[:])
    ones_sb = pool.tile([1, 512], BF16, tag="ones")
    nc.sync.dma_start(ones_sb[:], ones[:])

    loop_cm = tc.For_i(0, CFG["loop_n"], 1) if CFG["loop_n"] > 1 else None
    if loop_cm is not None:
        ctx.enter_context(loop_cm)

    # ---- big tag-chained tiles (creation order fixes slot reuse) --------
    xb_sb = [big.tile([128, NPOS], BF16, tag="big", name=f"xb_sb{h}") for h in range(2)]
    for h in range(2):
        for qt in range(4):
            qs = slice(qt * NPOS // 4, (qt + 1) * NPOS // 4)
            nc.sync.dma_start(xb_sb[h][:, qs], xb[h * 128:(h + 1) * 128, qs])
    k_sb = big.tile([128, NPOS], BF16, tag="big")
    q_sb = big.tile([128, NPOS], BF16, tag="big")
    v_sb = big.tile([128, NPOS], BF16, tag="big")

    # ---- projections (k/q/v chunk-interleaved, evicts alternate engines) -
    for ch in range(NCHUNK):
        sl = slice(ch * 512, (ch + 1) * 512)
        for j, (nm, dst, bias) in enumerate(
                (("wk", k_sb, "bk"), ("wq", q_sb, "bq"), ("wv", v_sb, "bv"))):
            ps = pp_mm.tile([128, 512], F32, tag="mm", name=f"pj{nm}{ch}")
            nc.tensor.matmul(ps[:], w_sb[nm, 0][:], xb_sb[0][:, sl],
                             start=True, stop=False)
            nc.tensor.matmul(ps[:], w_sb[nm, 1][:], xb_sb[1][:, sl],
                             start=False, stop=True)
            if (3 * ch + j) % 2:
                nc.vector.tensor_scalar_add(dst[:, sl], ps[:], b_sb[bias][:])
            else:
                nc.scalar.activation(dst[:, sl], ps[:], AF.Identity,
                                     bias=b_sb[bias][:], scale=1.0)

    def hw_slice(t, hw):
        # cols {d*256 + hw, d in 0..63} of a [128, NPOS] tile -> [128, 1, 64]
        return t[:].rearrange("p (d hw) -> p d hw", hw=HW)[
            :, :, hw:hw + 1].rearrange("p d hw -> p hw d")

    # ---- v^T via PE transpose: [128,64] -> [64,128], 4 hw per psum bank -
    vT_sb = big.tile([128, NPOS], BF16, tag="big")       # reuses xb0 slot
    for g in range(HW // 4):
        r0 = ((4 * g) // 128) * 64
        ps = pp_tr.tile([128, 512], BF16, tag="tr")
        for u in range(4):
            hw = 4 * g + u
            nc.tensor.matmul(ps[r0:r0 + 64, u * 128:(u + 1) * 128],
                             hw_slice(v_sb, hw), id_sb[:], is_transpose=True,
                             start=(u == 0), stop=(u == 3))
        cs = ((4 * g) % 128) * 128
        if g % 2:
            nc.scalar.copy(vT_sb[r0:r0 + 64, cs:cs + 512], ps[r0:r0 + 64, :])
        else:
            nc.vector.tensor_copy(vT_sb[r0:r0 + 64, cs:cs + 512],
                                  ps[r0:r0 + 64, :])

    aTT_sb = med.tile([128, 64 * 128], BF16, tag="aTT")
    att_sb = big.tile([128, NPOS], BF16, tag="big")      # 7th: reuses xb1 slot
    att_view = att_sb[:].rearrange("p (d hw) -> p d hw", hw=HW)

    # ---- out-projection for one hw half (strided pos slices) ------------
    def emit_out_half(half, d0=0, d1=16):
        hw0 = half * 128
        for dch in range(d0, d1):       # 4 d-values per chunk
            for h in range(2):
                ps = pp_mm.tile([128, 512], F32, tag="mm", name=f"o{half}{dch}{h}")
                rhs = att_view[:, 4 * dch:4 * dch + 4, hw0:hw0 + 128]
                nc.tensor.matmul(ps[:], woT_sb[:, h * 128:(h + 1) * 128],
                                 rhs, start=True, stop=False)
                nc.tensor.matmul(ps[:], boT_sb[:, h * 128:(h + 1) * 128],
                                 ones_sb[:], start=False, stop=True)
                xr = xring.tile([128, 512], F32, tag="xr", name=f"xr{half}{dch}{h}")
                xv = xf[h * 128:(h + 1) * 128, :].rearrange(
                    "p (d hw) -> p d hw", hw=HW)[:, 4 * dch:4 * dch + 4,
                                                 hw0:hw0 + 128]
                nc.sync.dma_start(xr[:], xv)
                ot = oring.tile([128, 512], F32, tag="out", name=f"ot{half}{dch}{h}")
                nc.vector.tensor_add(ot[:], ps[:], xr[:])
                ov = out_d[h * 128:(h + 1) * 128, :].rearrange(
                    "p (d hw) -> p d hw", hw=HW)[:, 4 * dch:4 * dch + 4,
                                                 hw0:hw0 + 128]
                nc.sync.dma_start(ov, ot[:])

    # ---- attention, batches of 16 hw pairs (32 hw) ----------------------
    for batch in range(8):
        denom = ring.tile([128, 16], F32, tag="denom")
        rcp = ring.tile([128, 16], F32, tag="rcp")
        aT_sb = ring.tile([128, 16 * 64], BF16, tag="aT")
        for g8 in range(2):                 # 2 groups of 8 pairs
            ps = pp_et.tile([128, 512], F32, tag="eT")
            for i8 in range(8):
                i = g8 * 8 + i8
                p = batch * 16 + i
                for u in range(2):
                    hw = 2 * p + u
                    nc.tensor.matmul(ps[u * 64:u * 64 + 64,
                                        i8 * 64:(i8 + 1) * 64],
                                     hw_slice(q_sb, hw), hw_slice(k_sb, hw),
                                     start=True, stop=True,
                                     skip_group_check=True)
            esl = slice(g8 * 512, (g8 + 1) * 512)
            nc.scalar.activation(aT_sb[:, esl], ps[:], AF.Exp, scale=SCALE)
            nc.vector.reduce_sum(
                out=denom[:, g8 * 8:(g8 + 1) * 8],
                in_=aT_sb[:, esl].rearrange("p (i f) -> p i f", i=8),
                axis=mybir.AxisListType.X)
            nc.vector.reciprocal(rcp[:, g8 * 8:(g8 + 1) * 8],
                                 denom[:, g8 * 8:(g8 + 1) * 8])
        for i in range(16):
            nc.gpsimd.tensor_scalar_mul(aT_sb[:, i * 64:(i + 1) * 64],
                                        aT_sb[:, i * 64:(i + 1) * 64],
                                        rcp[:, i:i + 1])
        # a^T -> a transposes: 4 pairs per psum bank
        for g in range(4):
            p0 = batch * 16 + 4 * g
            r0 = ((2 * p0) // 128) * 64
            ps = pp_tr.tile([128, 512], BF16, tag="tr")
            for u in range(4):
                i = 4 * g + u
                nc.tensor.matmul(ps[r0:r0 + 64, u * 128:(u + 1) * 128],
                                 aT_sb[:, i * 64:(i + 1) * 64], id_sb[:],
                                 is_transpose=True, start=(u == 0), stop=(u == 3))
            pl = (p0 % 64)
            if g % 2:
                nc.scalar.copy(aTT_sb[r0:r0 + 64, pl * 128:pl * 128 + 512],
                               ps[r0:r0 + 64, :])
            else:
                nc.vector.tensor_copy(aTT_sb[r0:r0 + 64, pl * 128:pl * 128 + 512],
                                      ps[r0:r0 + 64, :])
        # att matmuls: 8 hw per psum bank
        for g in range(4):
            hw0 = batch * 32 + 8 * g
            ps = pp_at.tile([128, 512], F32, tag="mm", name=f"at{batch}{g}")
            for u in range(8):
                hw = hw0 + u
                r0 = (hw // 128) * 64
                hl = hw % 128
                pl = (hw // 2) % 64
                nc.tensor.matmul(
                    ps[:, u * 64:(u + 1) * 64],
                    vT_sb[r0:r0 + 64, hl * 128:(hl + 1) * 128],
                    aTT_sb[r0:r0 + 64, pl * 128 + (hw % 2) * 64:
                           pl * 128 + (hw % 2) * 64 + 64],
                    start=(u == 0), stop=(u == 7))
            dst = att_view[:, :, hw0:hw0 + 8].rearrange("p d hw -> p hw d")
            if g % 2:
                nc.scalar.copy(dst, ps[:])
            else:
                nc.vector.tensor_copy(dst, ps[:])
        if batch == 3:
            emit_out_half(0)
        elif batch == 7:
            emit_out_half(1)


def build():
    key = tuple(sorted((k, v) for k, v in CFG.items() if k != "trace"))
    if key in _CACHE:
        return _CACHE[key]
    nc = bacc.Bacc("TRN2", target_bir_lowering=False, debug=False, num_devices=8)
    xb = nc.dram_tensor("xb", [C, NPOS], BF16, kind="ExternalInput")
    xf = nc.dram_tensor("xf", [C, NPOS], F32, kind="ExternalInput")
    wkT = nc.dram_tensor("wkT", [C, S], BF16, kind="ExternalInput")
    wqT = nc.dram_tensor("wqT", [C, S], BF16, kind="ExternalInput")
    wvT = nc.dram_tensor("wvT", [C, S], BF16, kind="ExternalInput")
    woT = nc.dram_tensor("woT", [S, C], BF16, kind="ExternalInput")
    bk = nc.dram_tensor("bk", [S, 1], F32, kind="ExternalInput")
    bq = nc.dram_tensor("bq", [S, 1], F32, kind="ExternalInput")
    bv = nc.dram_tensor("bv", [S, 1], F32, kind="ExternalInput")
    bo = nc.dram_tensor("bo", [C, 1], F32, kind="ExternalInput")
    ident = nc.dram_tensor("ident", [128, 128], BF16, kind="ExternalInput")
    boT = nc.dram_tensor("boT", [1, C], BF16, kind="ExternalInput")
    ones = nc.dram_tensor("ones", [1, 512], BF16, kind="ExternalInput")
    out_d = nc.dram_tensor("out", [C, NPOS], F32, kind="ExternalOutput")
    from contextlib import ExitStack
    with tile.TileContext(nc) as tc, ExitStack() as ctx:
        _emit(nc, tc, (xb, xf, wkT, wqT, wvT, woT, bk, bq, bv, bo, ident, boT, ones, out_d),
              ctx)
    nc.compile()
    _CACHE[key] = nc
    return nc


def make_in_maps(x, wk, bk, wq, bq, wv, bv, wo, bo):
    bf = ml_dtypes.bfloat16
    x = np.ascontiguousarray(np.asarray(x, dtype=np.float32)).reshape(B, C, NPOS)
    com = {
        "wkT": np.ascontiguousarray(np.asarray(wk, np.float32).T).astype(bf),
        "wqT": np.ascontiguousarray(np.asarray(wq, np.float32).T).astype(bf),
        "wvT": np.ascontiguousarray(np.asarray(wv, np.float32).T).astype(bf),
        "woT": np.ascontiguousarray(np.asarray(wo, np.float32).T).astype(bf),
        "bk": np.asarray(bk, np.float32).reshape(S, 1),
        "bq": np.asarray(bq, np.float32).reshape(S, 1),
        "bv": np.asarray(bv, np.float32).reshape(S, 1),
        "bo": np.asarray(bo, np.float32).reshape(C, 1),
        "ident": np.eye(128, dtype=bf),
        "boT": np.asarray(bo, np.float32).reshape(1, C).astype(bf),
        "ones": np.ones((1, 512), dtype=bf),
    }
    return [dict(com, xf=x[b], xb=x[b].astype(bf)) for b in range(B)]


def run(x, wk, bk, wq, bq, wv, bv, wo, bo, **kw):
    nc = build()
    maps = make_in_maps(x, wk, bk, wq, bq, wv, bv, wo, bo)
    res = run_bass_kernel_spmd(nc, maps, core_ids=list(range(B)), **kw)
    out = np.stack([np.asarray(r["out"]) for r in res.results])
    return out.reshape(B, C, D, H, W).astype(np.float32), res


def kernel(x, wk, bk, wq, bq, wv, bv, wo, bo):
    out, _ = run(x, wk, bk, wq, bq, wv, bv, wo, bo)
    return out



# revision 10
# speedup vs baseline: 58.9894x; 58.9894x over previous
"""Trainium2 Bass kernel for nn_Attention_layer (dense_transformer).

One batch element per NeuronCore (8 cores). Positions are hw-major:
pos = hw*64 + d, so per-hw [*, 64] slices and out-proj chunks are contiguous.

Host precompute folds all biases into exact linear algebra:
  x' = x + bo          (residual carries bo; projections corrected below)
  k = wk x' + (bk - wk bo)      q = wq x' + (bq - wq bo)
  vT'[pos,s] = (x'^T wv^T)[pos,s] = (wv x)[s,pos] + (wv bo)[s]   (no bias op)
  att' = vT'^T a = att_u + wv bo   (softmax cols sum to 1)
  att  = att' + (bv - wv bo)       (bias at att eviction)
  out  = wo att + x'               (single residual add, = reference exactly)

Per batch of 32 hw (2048 pos), software-pipelined across batches:
  PE: scores(b) -> proj k/q/vT'(b+1) -> aT->a transposes(b) -> att(b) -> out(b)
  Act: exp, q/att evictions (+bias), tr evictions, h1 residual adds
  DVE: denom reduce+rcp, k/vT evictions, h0 residual adds
  Pool(gpsimd): aT normalization (SBUF only)
  SP: all DMA (x loads, out stores; large contiguous transfers)
"""

import numpy as np
import ml_dtypes

import concourse.bacc as bacc
import concourse.tile as tile
from concourse import mybir
from concourse.bass_utils import run_bass_kernel_spmd

F32 = mybir.dt.float32
BF16 = mybir.dt.bfloat16
AF = mybir.ActivationFunctionType

B, C, S, D, H, W = 8, 256, 128, 64, 16, 16
HW = H * W              # 256
NPOS = HW * D           # 16384, pos = hw*64 + d
NBATCH = 8              # batches of 32 hw
BPOS = NPOS // NBATCH   # 2048 pos per batch
SCALE = float(1.0 / np.sqrt(np.float32(S)))

CFG = {
    "loop_n": 1,   # on-device repeats of the whole body (timing)
}

_CACHE = {}


def _emit(nc, tc, io, ctx):
    xb, wkT, wqT, wvT, woT, bk2, bq2, catt, ident, out_d = io

    # ---- pools ----------------------------------------------------------
    const = ctx.enter_context(tc.tile_pool(name="const", bufs=1))
    xpool = ctx.enter_context(tc.tile_pool(name="xpool", bufs=1))
    kqp = ctx.enter_context(tc.tile_pool(name="kqp", bufs=2))
    vp = ctx.enter_context(tc.tile_pool(name="vp", bufs=2))
    ap = ctx.enter_context(tc.tile_pool(name="ap", bufs=2))
    attp = ctx.enter_context(tc.tile_pool(name="attp", bufs=2))
    op = ctx.enter_context(tc.tile_pool(name="op", bufs=2))
    pp_kq = ctx.enter_context(tc.tile_pool(name="pp_kq", bufs=2, space="PSUM"))
    pp_v = ctx.enter_context(tc.tile_pool(name="pp_v", bufs=1, space="PSUM"))
    pp_ea = ctx.enter_context(tc.tile_pool(name="pp_ea", bufs=2, space="PSUM"))
    pp_tr = ctx.enter_context(tc.tile_pool(name="pp_tr", bufs=1, space="PSUM"))
    pp_out = ctx.enter_context(tc.tile_pool(name="pp_out", bufs=2, space="PSUM"))

    # ---- constants ------------------------------------------------------
    id_sb = const.tile([128, 128], BF16, tag="ident")
    nc.sync.dma_start(id_sb[:], ident[:])
    wk_sb, wq_sb, wv_sb = {}, {}, {}
    for h in range(2):
        sl = slice(h * 128, (h + 1) * 128)
        wk_sb[h] = const.tile([128, 128], BF16, tag=f"wk{h}", name=f"wk{h}")
        nc.sync.dma_start(wk_sb[h][:], wkT[sl, :])
        wq_sb[h] = const.tile([128, 128], BF16, tag=f"wq{h}", name=f"wq{h}")
        nc.sync.dma_start(wq_sb[h][:], wqT[sl, :])
        wv_sb[h] = const.tile([128, 128], BF16, tag=f"wv{h}", name=f"wv{h}")
        nc.sync.dma_start(wv_sb[h][:], wvT[sl, :])
    wo_sb = const.tile([128, 256], BF16, tag="wo")
    nc.sync.dma_start(wo_sb[:], woT[:])
    bk_sb = const.tile([128, 1], F32, tag="bk")
    nc.sync.dma_start(bk_sb[:], bk2[:])
    bq_sb = const.tile([128, 1], F32, tag="bq")
    nc.sync.dma_start(bq_sb[:], bq2[:])
    ca_sb = const.tile([128, 1], F32, tag="ca")
    nc.sync.dma_start(ca_sb[:], catt[:])

    loop_cm = tc.For_i(0, CFG["loop_n"], 1) if CFG["loop_n"] > 1 else None
    if loop_cm is not None:
        ctx.enter_context(loop_cm)

    # ---- x load (16 contiguous 0.5 MB DMAs) -----------------------------
    x_sb = [xpool.tile([128, NPOS], BF16, tag=f"x{h}", name=f"x_sb{h}")
            for h in range(2)]
    for ch in range(8):
        sl = slice(ch * 2048, (ch + 1) * 2048)
        for h in range(2):
            nc.sync.dma_start(x_sb[h][:, sl], xb[h * 128:(h + 1) * 128, sl])

    state = {}

    def emit_proj(b):
        bsl = slice(b * BPOS, (b + 1) * BPOS)
        k_t = kqp.tile([128, BPOS], BF16, tag="k", name=f"k{b}")
        q_t = kqp.tile([128, BPOS], BF16, tag="q", name=f"q{b}")
        vT_t = vp.tile([128, BPOS], BF16, tag="vT", name=f"vT{b}")
        state[b] = (k_t, q_t, vT_t)
        for nm, wsb, dst, bias in (("k", wk_sb, k_t, bk_sb),
                                   ("q", wq_sb, q_t, bq_sb)):
            for ch in range(4):
                csl = slice(b * BPOS + ch * 512, b * BPOS + (ch + 1) * 512)
                ps = pp_kq.tile([128, 512], F32, tag="kq", name=f"p{nm}{b}{ch}")
                nc.tensor.matmul(ps[:], wsb[0][:], x_sb[0][:, csl],
                                 start=True, stop=False)
                nc.tensor.matmul(ps[:], wsb[1][:], x_sb[1][:, csl],
                                 start=False, stop=True)
                nc.scalar.activation(dst[:, ch * 512:(ch + 1) * 512],
                                     ps[:], AF.Identity, bias=bias[:],
                                     scale=1.0)
        # vT': stationary = x' pos-block, streams wvT halves; block = 1 pair
        for g in range(4):
            ps = pp_v.tile([128, 512], F32, tag="v", name=f"pv{b}{g}")
            for u in range(4):
                p0 = (b * 16 + g * 4 + u) * 128
                for h in range(2):
                    nc.tensor.matmul(ps[:, u * 128:(u + 1) * 128],
                                     x_sb[h][:, p0:p0 + 128], wv_sb[h][:],
                                     start=(u == 0 and h == 0),
                                     stop=(u == 3 and h == 1))
            nc.vector.tensor_copy(vT_t[:, g * 512:(g + 1) * 512], ps[:])

    def emit_scores(b):
        k_t, q_t, _ = state[b]
        aT_t = ap.tile([128, 1024], BF16, tag="aT", name=f"aT{b}")
        den = ap.tile([128, 16], F32, tag="den", name=f"den{b}")
        rcp = ap.tile([128, 16], F32, tag="rcp", name=f"rcp{b}")
        state[b] += (aT_t,)
        for g in range(2):
            ps = pp_ea.tile([128, 512], F32, tag="ea", name=f"pe{b}{g}")
            for p8 in range(8):
                p = g * 8 + p8
                for u in range(2):
                    hw = (b * 16 + p) * 2 + u
                    csl = slice(hw * 64 - b * BPOS, hw * 64 - b * BPOS + 64)
                    nc.tensor.matmul(ps[u * 64:(u + 1) * 64,
                                        p8 * 64:(p8 + 1) * 64],
                                     q_t[:, csl], k_t[:, csl],
                                     start=(p8 == 0), stop=(p8 == 7),
                                     skip_group_check=True)
            esl = slice(g * 512, (g + 1) * 512)
            nc.scalar.activation(aT_t[:, esl], ps[:], AF.Exp, scale=SCALE)
            nc.vector.reduce_sum(
                out=den[:, g * 8:(g + 1) * 8],
                in_=aT_t[:, esl].rearrange("p (i f) -> p i f", i=8),
                axis=mybir.AxisListType.X)
            nc.vector.reciprocal(rcp[:, g * 8:(g + 1) * 8],
                                 den[:, g * 8:(g + 1) * 8])
        for p in range(16):
            nc.gpsimd.tensor_scalar_mul(aT_t[:, p * 64:(p + 1) * 64],
                                        aT_t[:, p * 64:(p + 1) * 64],
                                        rcp[:, p:p + 1])

    def emit_tail(b):
        bsl = slice(b * BPOS, (b + 1) * BPOS)
        _, _, vT_t, aT_t = state.pop(b)
        a_t = ap.tile([128, 1024], BF16, tag="a", name=f"a{b}")
        # aT -> a transposes: [64,64] blocks, even pair-halves at rows 0:64,
        # odd at 64:128 (matching vT' block layout)
        for g in range(2):
            # [128,1024] bf16 = full 2KB bank rows (zero-region alignment)
            ps = pp_tr.tile([128, 1024], BF16, tag="tr", name=f"pt{b}{g}")
            for p8 in range(8):
                p = g * 8 + p8
                for u in range(2):
                    r0 = u * 64
                    nc.tensor.matmul(ps[r0:r0 + 64, p8 * 64:(p8 + 1) * 64],
                                     aT_t[r0:r0 + 64, p * 64:(p + 1) * 64],
                                     id_sb[r0:r0 + 64, r0:r0 + 64],
                                     is_transpose=True,
                                     start=(p8 == 0), stop=(p8 == 7),
                                     skip_group_check=True)
            nc.scalar.copy(a_t[:, g * 512:(g + 1) * 512], ps[:, 0:512])
        # att: per hw, lhsT = vT' [64(i),128(s)], rhs = a [64(i),64(j)]
        att_t = attp.tile([128, BPOS], BF16, tag="att", name=f"att{b}")
        for g in range(4):
            ps = pp_ea.tile([128, 512], F32, tag="ea", name=f"pa{b}{g}")
            for u in range(8):
                p = (g * 8 + u) // 2
                osl = slice(u * 64, (u + 1) * 64)
                if u % 2 == 0:
                    # even hw: lhsT at rows 0:64, full-width out (tile (0,0))
                    nc.tensor.matmul(
                        ps[:, osl],
                        vT_t[0:64, p * 128:(p + 1) * 128],
                        a_t[0:64, p * 64:(p + 1) * 64],
                        start=(u == 0), stop=False, skip_group_check=True)
                else:
                    # odd hw: tile (64,0) with 128-wide out is illegal on HW;
                    # split into (64,0) and (64,64) quadrants
                    for sh in range(2):
                        nc.tensor.matmul(
                            ps[sh * 64:(sh + 1) * 64, osl],
                            vT_t[64:128, p * 128 + sh * 64: p * 128 + sh * 64 + 64],
                            a_t[64:128, p * 64:(p + 1) * 64],
                            start=False, stop=(u == 7 and sh == 1),
                            skip_group_check=True)
            nc.scalar.activation(att_t[:, g * 512:(g + 1) * 512], ps[:],
                                 AF.Identity, bias=ca_sb[:], scale=1.0)
        # out-proj + residual; h-major for stationary reuse
        for h in range(2):
            o_t = op.tile([128, BPOS], F32, tag=f"o{h}", name=f"o{b}{h}")
            for ch in range(4):
                sl = slice(ch * 512, (ch + 1) * 512)
                ps = pp_out.tile([128, 512], F32, tag="out", name=f"po{b}{h}{ch}")
                nc.tensor.matmul(ps[:], wo_sb[:, h * 128:(h + 1) * 128],
                                 att_t[:, sl], start=True, stop=True)
                xa = x_sb[h][:, b * BPOS + ch * 512: b * BPOS + (ch + 1) * 512]
                nc.vector.tensor_add(o_t[:, sl], ps[:], xa)
            nc.sync.dma_start(out_d[h * 128:(h + 1) * 128, bsl], o_t[:])

    emit_proj(0)
    for b in range(NBATCH):
        emit_scores(b)
        if b + 1 < NBATCH:
            emit_proj(b + 1)
        emit_tail(b)


def build():
    key = tuple(sorted(CFG.items()))
    if key in _CACHE:
        return _CACHE[key]
    nc = bacc.Bacc("TRN2", target_bir_lowering=False, debug=False, num_devices=8)
    xb = nc.dram_tensor("xb", [C, NPOS], BF16, kind="ExternalInput")
    wkT = nc.dram_tensor("wkT", [C, S], BF16, kind="ExternalInput")
    wqT = nc.dram_tensor("wqT", [C, S], BF16, kind="ExternalInput")
    wvT = nc.dram_tensor("wvT", [C, S], BF16, kind="ExternalInput")
    woT = nc.dram_tensor("woT", [S, C], BF16, kind="ExternalInput")
    bk2 = nc.dram_tensor("bk2", [S, 1], F32, kind="ExternalInput")
    bq2 = nc.dram_tensor("bq2", [S, 1], F32, kind="ExternalInput")
    catt = nc.dram_tensor("catt", [S, 1], F32, kind="ExternalInput")
    ident = nc.dram_tensor("ident", [128, 128], BF16, kind="ExternalInput")
    out_d = nc.dram_tensor("out", [C, NPOS], F32, kind="ExternalOutput")
    from contextlib import ExitStack
    with tile.TileContext(nc) as tc, ExitStack() as ctx:
        _emit(nc, tc, (xb, wkT, wqT, wvT, woT, bk2, bq2, catt, ident, out_d),
              ctx)
    nc.compile()
    _CACHE[key] = nc
    return nc


def make_in_maps(x, wk, bk, wq, bq, wv, bv, wo, bo):
    bf = ml_dtypes.bfloat16
    x = np.asarray(x, np.float32)
    wk, wq, wv, wo = (np.asarray(a, np.float32) for a in (wk, wq, wv, wo))
    bk, bq, bv, bo = (np.asarray(a, np.float32) for a in (bk, bq, bv, bo))
    # hw-major: pos = hw*64 + d
    xr = x.reshape(B, C, D, HW).transpose(0, 1, 3, 2)          # [B,C,HW,D]
    xp = (xr + bo[None, :, None, None]).reshape(B, C, NPOS).astype(bf)
    com = {
        "wkT": np.ascontiguousarray(wk.T).astype(bf),
        "wqT": np.ascontiguousarray(wq.T).astype(bf),
        "wvT": np.ascontiguousarray(wv.T).astype(bf),
        "woT": np.ascontiguousarray(wo.T).astype(bf),
        "bk2": (bk - wk @ bo).reshape(S, 1),
        "bq2": (bq - wq @ bo).reshape(S, 1),
        "catt": (bv - wv @ bo).reshape(S, 1),
        "ident": np.eye(128, dtype=bf),
    }
    return [dict(com, xb=np.ascontiguousarray(xp[b])) for b in range(B)]


def run(x, wk, bk, wq, bq, wv, bv, wo, bo, **kw):
    nc = build()
    maps = make_in_maps(x, wk, bk, wq, bq, wv, bv, wo, bo)
    res = run_bass_kernel_spmd(nc, maps, core_ids=list(range(B)), **kw)
    out = np.stack([np.asarray(r["out"]) for r in res.results])
    # [B, C, HW, D] -> [B, C, D, H, W]
    out = out.reshape(B, C, HW, D).transpose(0, 1, 3, 2)
    return np.ascontiguousarray(out).reshape(B, C, D, H, W), res


def kernel(x, wk, bk, wq, bq, wv, bv, wo, bo):
    out, _ = run(x, wk, bk, wq, bq, wv, bv, wo, bo)
    return out
